# revision 27
# baseline (speedup 1.0000x reference)
"""Trainium2 Bass kernel for nn_AMK_Block (dense transformer block).

Sequence-parallel across 8 NeuronCores: each core owns 512 of the 4096
rows. QKV + RoPE + RMS-norm computed locally feature-major in 2-head
[128, 512] tiles; K/V (with baked ones-columns for the attention row
sums) are all-gathered; the (elu+1)^2-kernel attention, W_o,
SwiGLU+depthwise-conv FFN and final norm run locally; per-core output
shards are concatenated on the host.

elu(x)+1 == min(exp(x), 1 + relu(x)), so
W = (elu(S)+1)^2 = min(exp(2S), (1+relu(S))^2). exp(2S) comes from ACT;
the rest is ONE fused custom-DVE op: minn(sq(relu(S)+1), e2).

v2 structure (vs the first working version):
- collective staging via DIRECT SBUF<->internal-DRAM DMAs (the external
  DRAM round-trip hops are gone; ~34MB less DRAM traffic per core).
- attention K/V loaded straight from the shared all-gather output with
  ONE batched DMA per (pair, K) and (pair, V) on the gpsimd queue.
- S for both heads of a pair lands in one [128, 1024] PSUM pair-tile
  (2 banks); exp and the custom DVE op each run ONCE per pair-tile.
- the attention tail copies Attr out of PSUM immediately (frees the
  accumulator banks for the next pair) and runs the norm broadcasts as
  float32r matmuls (1 cycle/row instead of 4).
- weight streams issue on the Activation HWDGE queue; W_down is fully
  prefetched into SBUF during attention.
- the conv halo exchange reads neighbour columns straight out of the
  shared collective buffer with 2 dynamically-addressed conditional
  DMAs (rank offsets fed per-core via a small uint32 input).
"""
import sys
import numpy as np

sys.path.insert(0, "/opt/trn_rl_repo")

import ml_dtypes  # noqa: E402
import concourse.bass as bass  # noqa: E402
import concourse.mybir as mybir  # noqa: E402
from concourse import tile  # noqa: E402
from concourse.bass import ds  # noqa: E402
from concourse.bass_utils import run_bass_kernel_spmd  # noqa: E402

BF16 = mybir.dt.bfloat16
F32 = mybir.dt.float32
F32R = mybir.dt.float32r
U32 = mybir.dt.uint32
AF = mybir.ActivationFunctionType
OP = mybir.AluOpType
bfdt = ml_dtypes.bfloat16

R = 8          # cores
N = 4096       # sequence
NL = N // R    # local rows = 512
D = 1024
H = 16
DH = 64
NK = D // 128  # 8 k-tiles of the d axis
INNER = 2816
CT = INNER // 128  # 22 channel tiles
K1 = 8 * DH * NL            # K elems per half (8 heads)
V1 = 4 * 128 * (8 * 65)     # V elems per half (4 row-tiles x 520 cols)
KV1 = K1 + V1
HALO = CT * 2 * 128         # boundary staging elems per rank
WCONV = NL + 2              # hf tile width (halo col + 512 + halo col)


# ------------------------------------------------------- custom DVE ops
def _register_dve_op(name, spec):
    """Append a custom DVE op to dve_ops.OPS at import time (the per-NEFF
    DVE table is generated from OPS at compile time; uops_sha is computed
    here and pinned so DveOp.compile's drift check passes)."""
    from concourse import dve_ops
    from concourse.dve_spec import lower as _lower, _has_src1
    from concourse.dve_uop import DveOpSpec

    for op in dve_ops.OPS:
        if op.name == name:
            return op
    row = dve_ops._CUSTOM_DVE_ROW_BASE + len(dve_ops.OPS)
    assert row < 0x20
    shas = {}
    for ver in ("v3", "v4"):
        d = DveOpSpec(name=name, opcode=row, uops=_lower(spec, ver=ver),
                      rd1_en=_has_src1(spec))
        shas[ver] = d.sha(ver)
    op = dve_ops.DveOp(name, spec, subdim=False, uops_sha=shas)
    dve_ops.OPS.append(op)
    dve_ops._SUB_OPCODE_FOR_NAME[name] = row
    dve_ops.CUSTOM_DVE_SPECS[name] = spec
    return op


def _make_ops():
    from concourse.dve_spec import Spec, Src0, Src1, C0, One, relu, sq, minn

    elu1sqmin = _register_dve_op(
        "ELU1SQMIN_ANT",
        Spec(
            body=minn(sq(relu(Src0) + One), Src1),
            reference=lambda in0, in1, s0, s1, imm2: np.minimum(
                np.square(np.maximum(in0, 0.0) + 1.0), in1
            ),
        ),
    )
    fma = _register_dve_op(
        "FMA_TT_ANT",
        Spec(
            body=Src0 * C0 + Src1,
            reference=lambda in0, in1, s0, s1, imm2: in0 * s0 + in1,
        ),
    )
    return elu1sqmin, fma


ELU1SQMIN, FMA_TT = _make_ops()


# ---------------------------------------------------------------- waitfix
def fix_sync_waits(nc, limit=1):
    """Walrus here allows at most 1 sem wait per instruction, and the
    runtime drops waits embedded in DMA instructions. Move excess waits
    onto same-engine NOPs inserted right before the instruction.

    Raw-ISA instructions (custom DVE ops) additionally lose their
    on_update semaphore increments, so for those move ALL waits to a
    leading NOP and all updates to a trailing NOP (engine queues are
    strict-FIFO, so the trailing NOP fires after the op completes)."""
    n_fixed = 0

    def make_nop(engine, hint):
        nop_bi = nc.engines[engine].nop(hint=hint)
        nop = nop_bi.ins if hasattr(nop_bi, "ins") else nop_bi
        cur = nc.cur_bb.bb
        cur_insts = list(cur.instructions)
        assert cur_insts and cur_insts[-1].name == nop.name
        cur.instructions = cur_insts[:-1]
        return nop

    for f in nc.m.functions:
        for bb in f.blocks:
            insts = list(bb.instructions)
            out = []
            changed = False
            for inst in insts:
                si = inst.sync_info
                tname = type(inst).__name__
                n_waits = len(si.on_wait) if (si is not None and si.on_wait) else 0
                n_upd = len(si.on_update) if (si is not None and si.on_update) else 0
                is_dma = "DMA" in tname or "DmaTranspose" in tname
                is_isa = ("ISA" in tname or "CustomDve" in tname) and "Load" not in tname
                eff = 0 if (is_dma or is_isa) else limit
                if n_waits > eff:
                    waits = list(si.on_wait)
                    keep = waits[-eff:] if eff > 0 else []
                    extra = waits[: len(waits) - eff]
                    for i in range(0, len(extra), limit):
                        ch = extra[i:i + limit]
                        nop = make_nop(inst.engine, "waitsplit")
                        nop.sync_info = mybir.SyncInfo(on_wait=list(ch), on_update=[])
                        out.append(nop)
                    inst.sync_info = mybir.SyncInfo(
                        on_wait=list(keep), on_update=list(si.on_update or [])
                    )
                    n_fixed += 1
                    changed = True
                out.append(inst)
            if changed:
                bb.instructions = out
    return n_fixed


# ---------------------------------------------------------------- build
def build_kernel():
    nc = bass.Bass()

    # register the non-default ACT scale/bias constants we use
    def reg_const(dtype, value):
        if (dtype, value) in nc.const_aps.aps:
            return
        t = nc.alloc_sbuf_tensor(f"const-{dtype.name}-{value}", [128, 1], dtype)
        nc.gpsimd.memset(t.ap(), value)
        nc.const_aps.aps[(dtype, value)] = t.ap()

    for v in (2.0, 1.0 / DH, 1.0 / D, 1e-5, DH * 1e-5, 1e-6):
        reg_const(F32, v)
    nc.all_engine_barrier()

    xt_e = nc.declare_dram_parameter("xt", [D, NL], BF16, isOutput=False)
    wqkv_e = nc.declare_dram_parameter("wqkv", [D, 3 * D], BF16, isOutput=False)
    wo_e = nc.declare_dram_parameter("wo", [D, D], BF16, isOutput=False)
    wup_e = nc.declare_dram_parameter("wup", [D, 2 * INNER], BF16, isOutput=False)
    wdn_e = nc.declare_dram_parameter("wdn", [INNER, D], BF16, isOutput=False)
    cs_e = nc.declare_dram_parameter("cs", [128, NL], BF16, isOutput=False)
    sn_e = nc.declare_dram_parameter("sn", [128, NL], BF16, isOutput=False)
    cw_e = nc.declare_dram_parameter("cw", [128, CT * 3], F32, isOutput=False)
    cb_e = nc.declare_dram_parameter("cb", [128, CT], F32, isOutput=False)
    sel2_e = nc.declare_dram_parameter("sel2", [2, 128], F32, isOutput=False)
    lsel_e = nc.declare_dram_parameter("lsel", [R, 1], BF16, isOutput=False)
    rsel_e = nc.declare_dram_parameter("rsel", [R, 1], BF16, isOutput=False)
    out_e = nc.declare_dram_parameter("out", [D, NL], BF16, isOutput=True)

    cc_kv_i = nc.dram_tensor("cc_kv_i", [2 * KV1], BF16)
    cc_kv_oA = nc.dram_tensor("cc_kv_oA", [R * KV1], BF16, addr_space="Shared")
    cc_kv_oB = nc.dram_tensor("cc_kv_oB", [R * KV1], BF16, addr_space="Shared")
    cc_hf_i = nc.dram_tensor("cc_hf_i", [HALO], BF16)
    cc_hf_o = nc.dram_tensor("cc_hf_o", [R * HALO], BF16, addr_space="Shared")
    GRP = [list(range(R))]

    with tile.TileContext(nc) as tc:
        per_ctx = tc.tile_pool(name="per", bufs=1)
        per = per_ctx.__enter__()
        psB_ctx = tc.tile_pool(name="psB", bufs=1, space="PSUM")
        psB = psB_ctx.__enter__()

        # ---------------- Ph0: loads + constants
        xb = []
        for k in range(NK):
            b = per.tile([128, NL], BF16, tag=f"xb{k}")
            nc.sync.dma_start(b[:], xt_e[128 * k:128 * (k + 1), :])
            xb.append(b)
        cs2 = per.tile([128, NL], BF16, tag="cs2")
        sn2 = per.tile([128, NL], BF16, tag="sn2")
        nc.sync.dma_start(cs2[:], cs_e[:])
        nc.sync.dma_start(sn2[:], sn_e[:])
        cw = per.tile([128, CT * 3], F32, tag="cw")
        cb = per.tile([128, CT], F32, tag="cb")
        nc.sync.dma_start(cw[:], cw_e[:])
        nc.sync.dma_start(cb[:], cb_e[:])
        # ss-matmul selector: col0 sums partitions 0:64, col1 64:128
        ones2sel = per.tile([128, 2], BF16, tag="o2sel")
        nc.vector.memset(ones2sel[:], 0.0)
        nc.vector.memset(ones2sel[0:64, 0:1], 1.0)
        nc.vector.memset(ones2sel[64:128, 1:2], 1.0)
        # bc-matmul selector: row0 -> partitions 0:64, row1 -> 64:128
        sel2 = per.tile([2, 128], F32, tag="sel2")
        nc.sync.dma_start(sel2[:], sel2_e[:])
        sel2b = per.tile([2, 128], BF16, tag="sel2b")
        nc.scalar.activation(sel2b[:], sel2[:], AF.Copy)
        ones1_128 = per.tile([1, 128], F32, tag="o1_128")
        nc.vector.memset(ones1_128[:], 1.0)
        ones128 = per.tile([128, 1], BF16, tag="o128")
        nc.vector.memset(ones128[:], 1.0)
        lsel = per.tile([R, 1], BF16, tag="lsel")
        rsel = per.tile([R, 1], BF16, tag="rsel")
        nc.sync.dma_start(lsel[:], lsel_e[:])
        nc.sync.dma_start(rsel[:], rsel_e[:])

        # W_down prefetch tiles (DMAs issued at attention start so they
        # stay off the startup critical path) — resident until Ph12
        wdn_kp = wdn_e[:].rearrange("(k p) c -> p k c", p=128)
        wdn_sb = []
        for mi in range(NK):
            w = per.tile([128, CT * 128], BF16, tag=f"wdn{mi}")
            wdn_sb.append(w)

        wqkv_kp = wqkv_e[:].rearrange("(k p) c -> p k c", p=128)

        # ---------------- pre-attention pools
        rp_ctx = tc.tile_pool(name="rp", bufs=3)
        rp = rp_ctx.__enter__()
        psQ_ctx = tc.tile_pool(name="psQ", bufs=3, space="PSUM")
        psQ = psQ_ctx.__enter__()
        psR_ctx = tc.tile_pool(name="psR", bufs=1, space="PSUM")
        psR = psR_ctx.__enter__()

        def rope_pair(col0, sqrt_scale, sqrt_bias, dst):
            """qkv matmul for a head pair + RoPE + per-head rms-norm;
            writes normalized [128, NL] bf16 into dst."""
            wh = rp.tile([128, NK * 128], BF16, tag="wh", bufs=3)
            nc.scalar.dma_start(
                wh[:].rearrange("p (k c) -> p k c", k=NK),
                wqkv_kp[:, :, col0:col0 + 128])
            ps = psQ.tile([128, NL], F32, tag="mm")
            for k in range(NK):
                nc.tensor.matmul(ps[:], wh[:, 128 * k:128 * (k + 1)], xb[k][:],
                                 start=(k == 0), stop=(k == NK - 1))
            raw = rp.tile([128, NL], BF16, tag="raw")
            nc.scalar.activation(raw[:], ps[:], AF.Copy)
            sw = rp.tile([128, NL], BF16, tag="sw")
            for (d0, s0) in ((0, 32), (32, 0), (64, 96), (96, 64)):
                nc.sync.dma_start(sw[d0:d0 + 32, :], raw[s0:s0 + 32, 0:NL])
            t1 = rp.tile([128, NL], BF16, tag="t1")
            nc.vector.tensor_mul(t1[:], raw[:], cs2[:])
            t2 = rp.tile([128, NL], BF16, tag="t2")
            nc.vector.tensor_mul(t2[:], sw[:], sn2[:])
            rot_ = rp.tile([128, NL], BF16, tag="rot")
            nc.vector.tensor_add(rot_[:], t1[:], t2[:])
            sq = rp.tile([128, NL], BF16, tag="sq")
            nc.gpsimd.tensor_mul(sq[:], rot_[:], rot_[:])
            ssp = psR.tile([2, NL], F32, tag="ss2")
            nc.tensor.matmul(ssp[:], ones2sel[:], sq[:], start=True, stop=True)
            sdp = rp.tile([2, NL], F32, tag="sdp")
            nc.scalar.activation(sdp[:], ssp[:], AF.Sqrt,
                                 scale=sqrt_scale, bias=sqrt_bias)
            rcp2 = rp.tile([2, NL], F32, tag="rcp2")
            nc.vector.reciprocal_approx_fast(rcp2[:], sdp[:])
            bc = psB.tile([128, NL], F32, tag="bc")
            nc.tensor.matmul(bc[:], sel2[:], rcp2[:], start=True, stop=True)
            nc.vector.tensor_mul(dst[:], rot_[:], bc[:])

        # ---------------- Ph1a: K pairs 0-3 (heads 0-7) -> cc_kv_i A
        cc_kv_iv = cc_kv_i.ap()

        def k_pair(u):
            kn = rp.tile([128, NL], BF16, tag="kn")
            rope_pair(D + 128 * u, 1.0 / DH, 1e-5, kn)
            half = u // 4
            hh = 2 * u - 8 * half
            base = half * KV1 + hh * DH * NL
            nc.sync.dma_start(
                cc_kv_iv[base:base + DH * NL].rearrange("(p n) -> p n", p=DH),
                kn[0:64, :])
            nc.sync.dma_start(
                cc_kv_iv[base + DH * NL:base + 2 * DH * NL]
                .rearrange("(p n) -> p n", p=DH),
                kn[64:128, :])

        for u in range(4):
            k_pair(u)

        # ---------------- Ph2: V row-major (with ones cols) -> cc_kv_i
        wvh = []
        for half in range(2):
            wv = rp.tile([128, NK * 512], BF16, tag=f"wv{half}", bufs=1)
            nc.scalar.dma_start(
                wv[:].rearrange("p (k c) -> p k c", k=NK),
                wqkv_kp[:, :, 2 * D + 512 * half:2 * D + 512 * (half + 1)])
            wvh.append(wv)
        for rt in range(4):
            va = rp.tile([128, H * 65], BF16, tag="vaug")
            vv = va[:].rearrange("p (h s) -> p h s", s=65)
            nc.vector.memset(vv[:, :, 64:65], 1.0)
            for half in range(2):
                ps = psQ.tile([128, NL], F32, tag="mm")
                for k in range(NK):
                    nc.tensor.matmul(ps[:], xb[k][:, 128 * rt:128 * (rt + 1)],
                                     wvh[half][:, 512 * k:512 * (k + 1)],
                                     start=(k == 0), stop=(k == NK - 1))
                dst = vv[:, 8 * half:8 * (half + 1), 0:64]
                src = ps[:].rearrange("p (h s) -> p h s", s=64)
                nc.scalar.activation(dst, src, AF.Copy)
            for half in range(2):
                vbase = half * KV1 + K1 + rt * 520 * 128
                nc.sync.dma_start(
                    cc_kv_iv[vbase:vbase + 520 * 128]
                    .rearrange("(p s) -> p s", p=128),
                    va[:, 520 * half:520 * (half + 1)])

        # ---------------- Ph3a: collective for half A (heads 0-7)
        nc.gpsimd.collective_compute(
            "AllGather", OP.bypass, replica_groups=GRP,
            ins=[cc_kv_i[0:KV1].opt()], outs=[cc_kv_oA.ap().opt()])

        # ---------------- Ph1b: K pairs 4-7 -> cc_kv_i B, then CC-B
        for u in range(4, 8):
            k_pair(u)
        nc.gpsimd.collective_compute(
            "AllGather", OP.bypass, replica_groups=GRP,
            ins=[cc_kv_i[KV1:2 * KV1].opt()], outs=[cc_kv_oB.ap().opt()])

        # ---------------- Ph4: Q side (overlaps collective)
        qn = []
        for u in range(8):
            q = per.tile([128, NL], BF16, tag=f"qn{u}")
            rope_pair(128 * u, 1.0, DH * 1e-5, q)
            qn.append(q)

        # ---------------- Ph5: V^T (feature-major, local rows) per pair
        vth = []
        vto = []
        for u in range(8):
            wvp = rp.tile([128, NK * 128], BF16, tag="wh", bufs=3)
            nc.scalar.dma_start(
                wvp[:].rearrange("p (k c) -> p k c", k=NK),
                wqkv_kp[:, :, 2 * D + 128 * u:2 * D + 128 * (u + 1)])
            vps = psQ.tile([128, NL], F32, tag="mm")
            for k in range(NK):
                nc.tensor.matmul(vps[:], wvp[:, 128 * k:128 * (k + 1)], xb[k][:],
                                 start=(k == 0), stop=(k == NK - 1))
            vv = per.tile([128, NL], BF16, tag=f"vth{u}")
            nc.scalar.activation(vv[:], vps[:], AF.Copy)
            vth.append(vv)
            vo = per.tile([64, NL], BF16, tag=f"vto{u}")
            nc.sync.dma_start(vo[:], vv[64:128, 0:NL])
            vto.append(vo)

        psR_ctx.__exit__(None, None, None)
        psQ_ctx.__exit__(None, None, None)
        rp_ctx.__exit__(None, None, None)

        # ---------------- Ph6: attention (head pairs)
        mp = []
        for u in range(8):
            p = per.tile([128, NL], BF16, tag=f"mp{u}")
            mp.append(p)
        attn_ctx = tc.tile_pool(name="attn", bufs=2)
        attn = attn_ctx.__enter__()
        psS_ctx = tc.tile_pool(name="psS", bufs=2, space="PSUM")
        psSp = psS_ctx.__enter__()
        psAT_ctx = tc.tile_pool(name="psAT", bufs=1, space="PSUM")
        psAT = psAT_ctx.__enter__()

        kvA_r = cc_kv_oA.ap().rearrange("(r b) -> r b", r=R)
        kvB_r = cc_kv_oB.ap().rearrange("(r b) -> r b", r=R)
        ksl = [None] * 8
        vsl = [None] * 8

        def attn_load(u):
            kv_r = kvA_r if u < 4 else kvB_r
            ul = u % 4
            koff = (2 * ul) * DH * NL
            kt = attn.tile([128, R * NL], BF16, tag="ksl", name=f"ksl{u}")
            nc.gpsimd.dma_start(
                kt[:].rearrange("p (r n) -> p r n", r=R),
                kv_r[:, koff:koff + 2 * DH * NL]
                .rearrange("r (j p n) -> (j p) r n", j=2, p=DH))
            ksl[u] = kt[:].rearrange("p (r n) -> p r n", r=R)
            voff = K1
            vt = attn.tile([128, R * 4 * 130], BF16, tag="vsl", name=f"vsl{u}")
            vtv = vt[:].rearrange("p (r c s) -> p r c s", r=R, c=4)
            for c in range(4):
                nc.gpsimd.dma_start(
                    vtv[:, :, c, :],
                    kv_r[:, voff + c * 520 * 128:voff + (c + 1) * 520 * 128]
                    .rearrange("r (p s) -> p r s", p=128)
                    [:, :, 130 * ul:130 * (ul + 1)])
            vsl[u] = vtv

        for mi in range(NK):
            nc.scalar.dma_start(
                wdn_sb[mi][:].rearrange("p (k c) -> p k c", k=CT),
                wdn_kp[:, :, 128 * mi:128 * (mi + 1)])
        attn_load(0)
        for u in range(8):
            if u < 7:
                attn_load(u + 1)
            at0 = psAT.tile([65, NL], F32, tag="attr0", name=f"at{u}_0")
            at1 = psAT.tile([65, NL], F32, tag="attr1", name=f"at{u}_1")
            for t in range(4 * R):
                r, c = t // 4, t % 4
                psS = psSp.tile([128, 2 * NL], F32, tag="psS")
                for j in range(2):
                    nc.tensor.matmul(
                        psS[:, NL * j:NL * (j + 1)],
                        ksl[u][64 * j:64 * (j + 1), r, 128 * c:128 * (c + 1)],
                        qn[u][64 * j:64 * (j + 1), :],
                        start=True, stop=True)
                e2 = attn.tile([128, 2 * NL], BF16, tag="e2")
                w2 = attn.tile([128, 2 * NL], BF16, tag="w2")
                import os as _os
                if _os.environ.get("KV_SPLIT_ELEM"):
                    for j in range(2):
                        sl = slice(NL * j, NL * (j + 1))
                        nc.scalar.activation(e2[:, sl], psS[:, sl],
                                             AF.Exp, scale=2.0)
                        nc.vector._custom_dve(ELU1SQMIN, out=w2[:, sl],
                                              in0=psS[:, sl], in1=e2[:, sl])
                else:
                    nc.scalar.activation(e2[:], psS[:], AF.Exp, scale=2.0)
                    nc.vector._custom_dve(ELU1SQMIN, out=w2[:], in0=psS[:],
                                          in1=e2[:])
                for j in range(2):
                    at = at0 if j == 0 else at1
                    nc.tensor.matmul(
                        at[:], vsl[u][:, r, c, 65 * j:65 * j + 65],
                        w2[:, NL * j:NL * (j + 1)],
                        start=(t == 0), stop=(t == 4 * R - 1))
            # tail: copy Attr out of PSUM, rowsum -> 1/(x+1e-6) ->
            # broadcast -> m = C/r - V
            aS0 = attn.tile([65, NL], BF16, tag="aS0")
            nc.scalar.activation(aS0[:], at0[:], AF.Copy)
            aS1 = attn.tile([65, NL], BF16, tag="aS1")
            nc.scalar.activation(aS1[:], at1[:], AF.Copy)
            rsp = attn.tile([2, NL], BF16, tag="rsp")
            nc.sync.dma_start(rsp[0:1, :], aS0[64:65, 0:NL])
            nc.sync.dma_start(rsp[1:2, :], aS1[64:65, 0:NL])
            rspf = attn.tile([2, NL], F32, tag="rspf")
            nc.vector.tensor_scalar(rspf[:], rsp[:], 1e-6, None, OP.add)
            rcp = attn.tile([2, NL], F32, tag="rcp")
            nc.vector.reciprocal_approx_fast(rcp[:], rspf[:])
            rcpb = attn.tile([2, NL], BF16, tag="rcpb")
            nc.scalar.activation(rcpb[:], rcp[:], AF.Copy)
            bcE = psB.tile([64, NL], F32, tag="bc", name=f"bcE{u}")
            nc.tensor.matmul(bcE[:], sel2b[:, 0:64], rcpb[:],
                             start=True, stop=True)
            bcsE = attn.tile([64, NL], BF16, tag="bcsE")
            nc.scalar.activation(bcsE[:], bcE[:], AF.Copy)
            bcO = psB.tile([64, NL], F32, tag="bc", name=f"bcO{u}")
            nc.tensor.matmul(bcO[:], sel2b[:, 64:128], rcpb[:],
                             start=True, stop=True)
            bcsO = attn.tile([64, NL], BF16, tag="bcsO")
            nc.scalar.activation(bcsO[:], bcO[:], AF.Copy)
            ccE = attn.tile([64, NL], BF16, tag="ccE")
            nc.vector.tensor_mul(ccE[:], aS0[0:64, :], bcsE[:])
            nc.gpsimd.tensor_sub(mp[u][0:64, :], ccE[:], vth[u][0:64, :])
            ccO = attn.tile([64, NL], BF16, tag="ccO")
            nc.vector.tensor_mul(ccO[:], aS1[0:64, :], bcsO[:])
            mO = attn.tile([64, NL], BF16, tag="mO")
            nc.gpsimd.tensor_sub(mO[:], ccO[:], vto[u][:])
            nc.sync.dma_start(mp[u][64:128, :], mO[0:64, 0:NL])

        psAT_ctx.__exit__(None, None, None)
        psS_ctx.__exit__(None, None, None)
        attn_ctx.__exit__(None, None, None)

        # ---------------- post-attention pools
        fp_ctx = tc.tile_pool(name="fp", bufs=2)
        fp = fp_ctx.__enter__()
        psF_ctx = tc.tile_pool(name="psF", bufs=2, space="PSUM")
        psF = psF_ctx.__enter__()
        psG_ctx = tc.tile_pool(name="psG", bufs=1, space="PSUM")
        psG = psG_ctx.__enter__()

        # ---------------- Ph8: W_o + residual + rms -> QI
        zt = []
        ss2 = psG.tile([1, NL], F32, tag="ss16")
        wo_kp = wo_e[:].rearrange("(k p) c -> p k c", p=128)
        for mi in range(NK):
            wom = fp.tile([128, NK * 128], BF16, tag="wom", bufs=3)
            nc.scalar.dma_start(
                wom[:].rearrange("p (k c) -> p k c", k=NK),
                wo_kp[:, :, 128 * mi:128 * (mi + 1)])
            ps = psF.tile([128, NL], F32, tag="mm")
            for k in range(NK):
                nc.tensor.matmul(ps[:], wom[:, 128 * k:128 * (k + 1)], mp[k][:],
                                 start=(k == 0), stop=(k == NK - 1))
            z = per.tile([128, NL], BF16, tag=f"zf{mi}")
            nc.vector.tensor_add(z[:], ps[:], xb[mi][:])
            zt.append(z)
            sq = fp.tile([128, NL], BF16, tag="sq2")
            nc.vector.tensor_mul(sq[:], z[:], z[:])
            nc.tensor.matmul(ss2[:], ones128[:], sq[:],
                             start=(mi == 0), stop=(mi == NK - 1))
        sd2 = fp.tile([1, NL], F32, tag="sd")
        nc.scalar.activation(sd2[:], ss2[:], AF.Sqrt, scale=1.0 / D, bias=1e-5)
        rc2 = fp.tile([1, NL], F32, tag="rc")
        nc.vector.reciprocal_approx_fast(rc2[:], sd2[:])
        bc2 = psB.tile([128, NL], F32, tag="bc", name="bcQI")
        nc.tensor.matmul(bc2[:], ones1_128[:], rc2[:], start=True, stop=True)
        qib = []
        for mi in range(NK):
            qb = per.tile([128, NL], BF16, tag=f"qib{mi}")
            nc.vector.tensor_mul(qb[:], zt[mi][:], bc2[:])
            qib.append(qb)

        # ---------------- Ph9: SwiGLU FFN up + Hf
        hfall = per.tile([128, CT * WCONV], BF16, tag="hfall")
        hfv = hfall[:].rearrange("p (c w) -> p c w", w=WCONV)
        wup_kp = wup_e[:].rearrange("(k p) c -> p k c", p=128)
        for c in range(CT):
            wgm = fp.tile([128, NK * 128], BF16, tag="wgm", bufs=3)
            nc.scalar.dma_start(
                wgm[:].rearrange("p (k c) -> p k c", k=NK),
                wup_kp[:, :, 128 * c:128 * (c + 1)])
            gps = psF.tile([128, NL], F32, tag="mm")
            for k in range(NK):
                nc.tensor.matmul(gps[:], wgm[:, 128 * k:128 * (k + 1)], qib[k][:],
                                 start=(k == 0), stop=(k == NK - 1))
            gs = fp.tile([128, NL], BF16, tag="gs")
            nc.scalar.activation(gs[:], gps[:], AF.Silu)
            wum = fp.tile([128, NK * 128], BF16, tag="wum", bufs=3)
            nc.scalar.dma_start(
                wum[:].rearrange("p (k c) -> p k c", k=NK),
                wup_kp[:, :, INNER + 128 * c:INNER + 128 * (c + 1)])
            ups = psF.tile([128, NL], F32, tag="mm")
            for k in range(NK):
                nc.tensor.matmul(ups[:], wum[:, 128 * k:128 * (k + 1)], qib[k][:],
                                 start=(k == 0), stop=(k == NK - 1))
            nc.vector.tensor_mul(hfv[:, c, 1:NL + 1], gs[:], ups[:])

        # stage boundary cols (layout [c][side][p]) and all-gather them
        cc_hf_iv = cc_hf_i.ap().rearrange("(c s p) -> p c s", s=2, p=128)
        nc.sync.dma_start(cc_hf_iv[:, :, 0:1], hfv[:, :, 1:2])
        nc.sync.dma_start(cc_hf_iv[:, :, 1:2], hfv[:, :, NL:NL + 1])
        nc.gpsimd.collective_compute(
            "AllGather", OP.bypass, replica_groups=GRP,
            ins=[cc_hf_i.ap().opt()], outs=[cc_hf_o.ap().opt()])

        # ---------------- Ph10: halo fixup — gather all ranks' boundary
        # cols, pick the neighbours with lsel/rsel one-hot matmuls
        hsbc = fp.tile([R, HALO], BF16, tag="hsbc", bufs=1)
        nc.sync.dma_start(
            hsbc[:], cc_hf_o.ap().rearrange("(r e) -> r e", r=R))
        hsv = hsbc[:].rearrange("r (c s p) -> r c s p", s=2, p=128)
        for side in range(2):
            sel = lsel if side == 0 else rsel
            hrow = fp.tile([1, CT * 128], BF16, tag=f"hrow{side}",
                           bufs=1)
            for ch in range(6):
                c0, c1 = 4 * ch, min(4 * (ch + 1), CT)
                if c0 >= c1:
                    break
                hps = psB.tile([1, 512], F32, tag="bc",
                               name=f"hps{side}_{ch}")
                nc.tensor.matmul(
                    hps[:, 0:128 * (c1 - c0)],
                    sel[:], hsv[:, c0:c1, 1 - side:2 - side, :],
                    start=True, stop=True)
                nc.scalar.activation(hrow[:, 128 * c0:128 * c1],
                                     hps[:, 0:128 * (c1 - c0)], AF.Copy)
            dstc = 0 if side == 0 else NL + 1
            for c in range(CT):
                eng = nc.sync if c % 2 == 0 else nc.scalar
                eng.dma_start(hfv[:, c, dstc:dstc + 1],
                              hrow[0:1, 128 * c:128 * (c + 1)])
        # zero the halo cols of the edge ranks (lsel/rsel rows are zero
        # there, so the matmul already returns 0 — nothing else needed)

        # ---------------- Ph11: depthwise conv + silu (custom FMA)
        for c in range(CT):
            base = c * WCONV
            a = fp.tile([128, NL], BF16, tag="cva")
            nc.vector.tensor_scalar_mul(a[:], hfall[:, base:base + NL],
                                        cw[:, 3 * c:3 * c + 1])
            b = fp.tile([128, NL], BF16, tag="cvb")
            nc.vector._custom_dve(FMA_TT, out=b[:],
                                  in0=hfall[:, base + 1:base + NL + 1],
                                  in1=a[:], s0=cw[:, 3 * c + 1:3 * c + 2])
            d = fp.tile([128, NL], BF16, tag="cvd")
            nc.vector._custom_dve(FMA_TT, out=d[:],
                                  in0=hfall[:, base + 2:base + NL + 2],
                                  in1=b[:], s0=cw[:, 3 * c + 2:3 * c + 3])
            nc.scalar.activation(hfall[:, base + 1:base + NL + 1], d[:],
                                 AF.Silu, bias=cb[:, c:c + 1])

        # ---------------- Ph12: W_down + residual + final rms -> out
        ft = []
        ss3 = psG.tile([1, NL], F32, tag="ss16")
        for mi in range(NK):
            ps = psF.tile([128, NL], F32, tag="mm")
            for c in range(CT):
                nc.tensor.matmul(ps[:], wdn_sb[mi][:, 128 * c:128 * (c + 1)],
                                 hfall[:, c * WCONV + 1:c * WCONV + NL + 1],
                                 start=(c == 0), stop=(c == CT - 1))
            fz = per.tile([128, NL], BF16, tag=f"zf{mi}")
            nc.vector.tensor_add(fz[:], ps[:], qib[mi][:])
            ft.append(fz)
            sq = fp.tile([128, NL], BF16, tag="sq2")
            nc.vector.tensor_mul(sq[:], fz[:], fz[:])
            nc.tensor.matmul(ss3[:], ones128[:], sq[:],
                             start=(mi == 0), stop=(mi == NK - 1))
        sd3 = fp.tile([1, NL], F32, tag="sd")
        nc.scalar.activation(sd3[:], ss3[:], AF.Sqrt, scale=1.0 / D, bias=1e-5)
        rc3 = fp.tile([1, NL], F32, tag="rc")
        nc.vector.reciprocal_approx_fast(rc3[:], sd3[:])
        bc3 = psB.tile([128, NL], F32, tag="bc", name="bcFin")
        nc.tensor.matmul(bc3[:], ones1_128[:], rc3[:], start=True, stop=True)
        for mi in range(NK):
            o = fp.tile([128, NL], BF16, tag="ofin")
            nc.vector.tensor_mul(o[:], ft[mi][:], bc3[:])
            nc.sync.dma_start(out_e[128 * mi:128 * (mi + 1), :], o[:])

        psG_ctx.__exit__(None, None, None)
        psF_ctx.__exit__(None, None, None)
        fp_ctx.__exit__(None, None, None)
        psB_ctx.__exit__(None, None, None)
        per_ctx.__exit__(None, None, None)

    fix_sync_waits(nc)
    # populate .instr bytes for extended-inst ISA subclasses (custom DVE
    # ops) — without this walrus codegen fails with "ISA wrong length"
    from concourse.library_overlay import lower_extended_insts
    lower_extended_insts(nc)
    return nc


_NC = None
_LAST_RES = None


def make_in_maps(Q_in, cos, sin, W_qkv, W_o, W_up, conv_w, conv_b, W_down):
    Q_in = np.asarray(Q_in, dtype=np.float32)
    cos = np.asarray(cos, dtype=np.float32)
    sin = np.asarray(sin, dtype=np.float32)
    wqkv = np.ascontiguousarray(np.asarray(W_qkv, np.float32).astype(bfdt))
    wo = np.ascontiguousarray(np.asarray(W_o, np.float32).astype(bfdt))
    wup = np.ascontiguousarray(np.asarray(W_up, np.float32).astype(bfdt))
    wdn = np.ascontiguousarray(np.asarray(W_down, np.float32).astype(bfdt))
    cwt = np.asarray(conv_w, np.float32)[:, 0, :].T  # [2816, 3]
    cw = np.ascontiguousarray(
        cwt.reshape(CT, 128, 3).transpose(1, 0, 2).reshape(128, CT * 3))
    cb = np.ascontiguousarray(np.asarray(conv_b, np.float32).reshape(CT, 128).T)

    in_maps = []
    for c in range(R):
        rows = slice(NL * c, NL * (c + 1))
        xt = np.ascontiguousarray(Q_in[0, rows, :].T.astype(bfdt))
        csA = cos[rows, 0:32].T.astype(bfdt)      # [32, NL]
        snA = sin[rows, 0:32].T.astype(bfdt)
        csf = np.ascontiguousarray(np.concatenate([csA, csA, csA, csA], axis=0))
        snf = np.ascontiguousarray(np.concatenate([-snA, snA, -snA, snA], axis=0))
        s2 = np.zeros((2, 128), np.float32)
        s2[0, 0:64] = 1.0
        s2[1, 64:128] = 1.0
        ls = np.zeros((R, 1), bfdt)
        rs = np.zeros((R, 1), bfdt)
        if c > 0:
            ls[c - 1, 0] = 1.0
        if c < R - 1:
            rs[c + 1, 0] = 1.0
        in_maps.append({
            "xt": xt, "wqkv": wqkv, "wo": wo, "wup": wup, "wdn": wdn,
            "cs": csf, "sn": snf, "cw": cw, "cb": cb, "sel2": s2,
            "lsel": ls, "rsel": rs,
        })
    return in_maps


def kernel(Q_in, cos, sin, W_qkv, W_o, W_up, conv_w, conv_b, W_down):
    global _NC
    if _NC is None:
        _NC = build_kernel()
    nc = _NC
    in_maps = make_in_maps(Q_in, cos, sin, W_qkv, W_o, W_up, conv_w,
                           conv_b, W_down)

    import os
    trace = bool(os.environ.get("KTRACE"))
    res = run_bass_kernel_spmd(nc, in_maps, core_ids=list(range(R)), trace=trace)
    global _LAST_RES
    _LAST_RES = res
    if trace:
        print(f"HW exec time: {res.exec_time_ns} ns")
    out = np.empty((1, N, D), np.float32)
    for c in range(R):
        out[0, NL * c:NL * (c + 1), :] = (
            np.asarray(res.results[c]["out"]).astype(np.float32).T)
    return out
